# revision 1
# baseline (speedup 1.0000x reference)
"""GraphSAGE 3-layer GNN on 8 TRN2 NeuronCores.

Strategy (node-sharded, feature-replicated):
  - 50000 nodes padded to 50176 = 8 cores x 49 blocks x 128. Core c owns
    destination rows [6272c, 6272c+6272).
  - Per layer, every core gathers h[src] rows for its local edges with
    gpsimd.dma_gather (one 512B/256B row per edge, landing on one SBUF
    partition), then segment-sums them per 128-dst block with a one-hot
    (x 1/deg) matmul on TensorE:  meanT[feat,dst] += G[e,feat].T @ S[e,dst].
    S (one-hot x 1/deg) and the int16 gather indices are host-precomputed
    and SBUF-resident; the node space is split at HALF (host-optimized)
    so indices fit int16, giving a lo and a hi gather per supergroup,
    each further split in two and spread over the 4 SWDGE queues so all
    4 Q7 core pairs generate DMA descriptors concurrently (the gather
    stream, ~3.3 ns/edge, is the kernel's critical path).
  - Dense SAGE terms run in transposed orientation so tanh bias is
    per-partition:  outT[feat_out, node] = Wl @ meanT + Wr @ hT (+bias);
    hT (feature-major current features) stays SBUF-resident per block.
  - Dropout masks are precomputed on the host as {0, 1.25} multipliers.
  - One AllGather per layer boundary replicates the node-major output
    (via PE transpose) for the next layer's gathers. Layer 3 aggregates
    the W3l-projected features p = h2 @ W3l^T (128 dims), so only p is
    gathered; h2 stays local for the W3r term.
  - Emission is software-pipelined two-cursor: gathers/masks for
    supergroup k+LOOK are emitted before compute of supergroup k so the
    in-order engine queues never serialize consecutive supergroups.

kernel(**inputs) takes full-size numpy inputs, does all index/layout prep on
the host, compiles one SPMD Bass program, runs it on cores 0-7, and returns
the full [50000, 128] float32 output.
"""

import math
from contextlib import ExitStack

import ml_dtypes
import numpy as np

import concourse.bass as bass
import concourse.bacc as bacc
import concourse.mybir as mybir
import concourse.tile as tile
from concourse.bass_utils import run_bass_kernel_spmd
from concourse.masks import make_identity

P = 128
NCORES = 8
DROP_P = 0.2
BF16 = ml_dtypes.bfloat16

F32 = mybir.dt.float32
BF16_T = mybir.dt.bfloat16
I16 = mybir.dt.int16
FP8_T = mybir.dt.float8e4
FP8 = ml_dtypes.float8_e4m3


class Plan:
    pass


# ----------------------------------------------------------------------------
# Planning (shared across cores; the SPMD program structure depends on it)
# ----------------------------------------------------------------------------

def make_plan(n_nodes, n_edges, d_in, d_out, edge_index, gmax=12):
    pl = Plan()
    pl.N = n_nodes
    pl.E = n_edges
    pl.D = d_in
    pl.DO = d_out
    pl.KB = d_in // P
    assert d_in % P == 0 and d_out == P
    pl.NB = math.ceil(n_nodes / NCORES / P)     # blocks per core
    pl.NPC = pl.NB * P                          # padded nodes per core
    pl.NPAD = NCORES * pl.NPC

    src = np.asarray(edge_index[0], dtype=np.int64)
    dst = np.asarray(edge_index[1], dtype=np.int64)
    assert src.min() >= 0 and src.max() < n_nodes
    deg = np.bincount(dst, minlength=n_nodes)
    pl.invdeg = (1.0 / np.maximum(deg, 1)).astype(np.float32)

    core = dst // pl.NPC
    lb = (dst - core * pl.NPC) // P             # dst block within core
    pl.dst_local = (dst - core * pl.NPC) % P
    pl.core = core
    pl.lb = lb
    pl.src = src
    pl.dst = dst

    # Shared h tables use an AllGather-chunk-major layout: each layer's
    # AllGather is split in two (blocks [0,S1B) then [S1B,NB)) so the
    # first chunk's transfer overlaps the tail of the layer's compute,
    # and the second is hidden behind the next layer's lo-piece gathers.
    # The lo/hi gather-piece boundary coincides with the chunk boundary,
    # so each piece gathers from its own (single-writer) Shared tensor.
    # Node (core c, local i) sits at position:
    #   i <  r0:  c*r0 + i                       (chunk a, piece lo)
    #   i >= r0:  8*r0 + c*r1 + (i - r0)         (chunk b, piece hi)
    # S1B must keep both pieces' index windows inside int16 and is chosen
    # to minimize total padded chunks.
    if pl.NPAD <= 32768:
        npiece = 1
        pl.S1B = pl.NB
        best = None
    else:
        npiece = 2
        loc = src % pl.NPC
        s_lo = -(-(pl.NPAD - 32768) // (NCORES * P))     # piece-hi window
        s_hi = 32768 // (NCORES * P)                     # piece-lo window
        assert s_lo <= s_hi and s_hi >= 1
        best = None
        key_base = (core * pl.NB + lb) * 2
        for s1b in range(s_lo, min(s_lo + 3, s_hi, pl.NB - 1) + 1):
            is_hi = (loc >= s1b * P).astype(np.int64)
            cnt = np.bincount(key_base + is_hi,
                              minlength=NCORES * pl.NB * 2)
            tot = int((-(-cnt.reshape(NCORES, pl.NB, 2).max(axis=0)
                         // P)).sum())
            if best is None or tot < best[0]:
                best = (tot, s1b)
        pl.S1B = best[1]
    r0 = pl.S1B * P
    r1 = pl.NPC - r0
    pl.AG_R0 = r0
    pl.AG_BASE1 = NCORES * r0
    nodes = np.arange(n_nodes)
    nc_, ni = nodes // pl.NPC, nodes % pl.NPC
    pl.pad_of_node = np.where(
        ni < r0, nc_ * r0 + ni, pl.AG_BASE1 + nc_ * r1 + (ni - r0))
    srcpos = pl.pad_of_node[src]
    pl.HALF = pl.AG_BASE1 if npiece == 2 else pl.NPAD
    pl.src_q = (srcpos >= pl.HALF).astype(np.int64)
    pl.src_idx = np.where(pl.src_q == 1, srcpos - pl.HALF, srcpos)
    pl.NPIECE = npiece
    pl.gpb = np.array([0, pl.HALF, pl.NPAD][:npiece + 1], dtype=np.int64)
    assert pl.src_idx.max() < 32768

    # chunk caps per (dst block, src piece): max need over cores
    key = (core * pl.NB + lb) * npiece + pl.src_q
    cnt = np.bincount(key, minlength=NCORES * pl.NB * npiece)
    cnt = cnt.reshape(NCORES, pl.NB, npiece)
    pl.nchunk_bq = -(-cnt.max(axis=0) // P)    # [NB, npiece]

    # supergroups: consecutive blocks, per-piece chunk sums <= gmax
    sgs = []
    cur, s_q = [], np.zeros(npiece, dtype=np.int64)
    for b in range(pl.NB):
        if cur and (s_q + pl.nchunk_bq[b] > gmax).any():
            sgs.append(cur)
            cur, s_q = [], np.zeros(npiece, dtype=np.int64)
        cur.append(b)
        s_q = s_q + pl.nchunk_bq[b]
    if cur:
        sgs.append(cur)

    # global chunk ordering: per sg -> per piece -> blocks in order
    pl.sgs = []
    cg = 0
    pl.block_chunks = [[] for _ in range(pl.NB)]  # (sg, piece, slot, cg)
    for si, blocks in enumerate(sgs):
        info = {"blocks": blocks, "q_start": [], "q_n": []}
        for q in range(npiece):
            start = cg
            for b in blocks:
                for _ in range(pl.nchunk_bq[b][q]):
                    pl.block_chunks[b].append((si, q, cg - start, cg))
                    cg += 1
            info["q_start"].append(start)
            info["q_n"].append(cg - start)
        pl.sgs.append(info)
    pl.NCHUNK = cg
    pl.GMAX = max(max(sg["q_n"]) for sg in pl.sgs)
    pl.SMAX = max(sum(sg["q_n"]) for sg in pl.sgs)
    return pl


# ----------------------------------------------------------------------------
# Per-core host packing
# ----------------------------------------------------------------------------

def _featT(arr, KB):
    """[n, KB*128] -> [128, KB, n]  (feature-major layout)."""
    n = arr.shape[0]
    return np.ascontiguousarray(arr.T.reshape(KB, P, n).transpose(1, 0, 2))


def pack_core(pl, c, x_padded, scale1, scale2):
    NB, NPC, NCHUNK, NQ = pl.NB, pl.NPC, pl.NCHUNK, pl.NPIECE
    mine = pl.core == c
    sidx, q, lb, dl = (pl.src_idx[mine], pl.src_q[mine],
                       pl.lb[mine], pl.dst_local[mine])
    w = pl.invdeg[pl.dst[mine]]

    order = np.lexsort((q, lb))
    sidx, q, lb, dl, w = (a[order] for a in (sidx, q, lb, dl, w))

    # chunk start (cg) per (block, piece)
    cg_start = np.zeros((NB, NQ), dtype=np.int64)
    for b in range(NB):
        for (si, qq, sl, cg) in pl.block_chunks[b]:
            if cg_start[b, qq] == 0 or sl == 0:
                pass
        firsts = {}
        for (si, qq, sl, cg) in pl.block_chunks[b]:
            if qq not in firsts:
                firsts[qq] = cg
        for qq, cg0 in firsts.items():
            cg_start[b, qq] = cg0

    key = lb * NQ + q
    grp_first = np.searchsorted(key, np.arange(NB * NQ))
    rank = np.arange(len(sidx)) - grp_first[key]
    assert (rank < pl.nchunk_bq[lb, q] * P).all(), "chunk overflow"
    pos = cg_start[lb, q] * P + rank

    idx_vals = np.zeros(NCHUNK * P, dtype=np.int16)
    idx_vals[pos] = sidx.astype(np.int16)

    # 0/1 one-hot scatter matrix in fp8 (exact); the 1/deg mean scaling is
    # applied post-sum from invdegT, so S stays 8.9MB instead of 17.7MB
    S01 = np.zeros((P, NCHUNK, P), dtype=FP8)
    S01[pos % P, pos // P, dl] = 1.0

    # per-destination 1/deg, feature-broadcast: invdegT[p, b, d] for the
    # core's dst node (b, d), identical on every partition p
    idg = np.ones((NB * P,), dtype=np.float32)
    lo_node = c * NPC
    n_real = min(NPC, pl.N - lo_node)
    idg[:n_real] = pl.invdeg[lo_node:lo_node + n_real]
    invdegT = np.broadcast_to(
        idg.reshape(1, NB, P), (P, NB, P)).astype(BF16)

    # layer-1 pre-gathered edge features (host-side index copy of x): slot
    # pos holds x[src] so layer 1 streams contiguous DMAs instead of
    # per-row SWDGE gathers
    srcg = pl.src[mine][order]
    pg1 = np.zeros((P, NCHUNK, pl.D), dtype=FP8)
    pg1[pos % P, pos // P, :] = x_padded[srcg].astype(FP8)

    # wrap idx into [128, NCHUNK*8] int16, per-gather-call 16-element wrap
    idx_all = np.zeros((P, NCHUNK * 8), dtype=np.int16)
    for sg in pl.sgs:
        for qq in range(NQ):
            s0, n = sg["q_start"][qq], sg["q_n"][qq]
            if n == 0:
                continue
            seg = idx_vals[s0 * P:(s0 + n) * P]
            blk = seg.reshape(-1, 16).T
            idx_all[:, s0 * 8:(s0 + n) * 8] = np.tile(blk, (8, 1))

    lo_node, hi_node = c * NPC, min((c + 1) * NPC, pl.N)
    nreal = hi_node - lo_node
    xo = np.zeros((NPC, pl.D), dtype=BF16)
    xo[:nreal] = x_padded[lo_node:hi_node]
    m1 = np.zeros((NPC, pl.D), dtype=np.float32)
    m2 = np.zeros((NPC, pl.D), dtype=np.float32)
    m1[:nreal] = scale1[lo_node:hi_node]
    m2[:nreal] = scale2[lo_node:hi_node]

    return {
        "s01": S01,
        "invdegT": np.ascontiguousarray(invdegT),
        "idx_all": idx_all,
        "pg1": pg1,
        "xT": _featT(xo, pl.KB).astype(BF16),
        "m1T": _featT(m1, pl.KB).astype(BF16),
        "m2T": _featT(m2, pl.KB).astype(BF16),
    }


# ----------------------------------------------------------------------------
# Bass program
# ----------------------------------------------------------------------------

def build_program(pl, n_cores=NCORES, g_bufs=3, look=2, boost=1):
    D, DO, KB, NB, NPC, NPAD = pl.D, pl.DO, pl.KB, pl.NB, pl.NPC, pl.NPAD
    NCHUNK, GMAX, SMAX, NQ = pl.NCHUNK, pl.GMAX, pl.SMAX, pl.NPIECE
    Tanh = mybir.ActivationFunctionType.Tanh

    nc = bacc.Bacc("TRN2", target_bir_lowering=False, debug=False,
                   num_devices=n_cores, dynamic_dma_scratch_size=8192,
                   num_swdge_queues=4)

    # ---- DRAM parameters
    pg1_d = nc.dram_tensor("pg1", [P, NCHUNK, D], FP8_T, kind="ExternalInput")
    xT_d = nc.dram_tensor("xT", [P, KB, NPC], BF16_T, kind="ExternalInput")
    m1_d = nc.dram_tensor("m1T", [P, KB, NPC], BF16_T, kind="ExternalInput")
    m2_d = nc.dram_tensor("m2T", [P, KB, NPC], BF16_T, kind="ExternalInput")
    s01_d = nc.dram_tensor("s01", [P, NCHUNK, P], FP8_T, kind="ExternalInput")
    idg_d = nc.dram_tensor("invdegT", [P, NB, P], BF16_T, kind="ExternalInput")
    idx_d = nc.dram_tensor("idx_all", [P, NCHUNK * 8], I16, kind="ExternalInput")
    w1l_d = nc.dram_tensor("w1l", [P, KB, D], BF16_T, kind="ExternalInput")
    w1r_d = nc.dram_tensor("w1r", [P, KB, D], BF16_T, kind="ExternalInput")
    w2l_d = nc.dram_tensor("w2l", [P, KB, D], BF16_T, kind="ExternalInput")
    w2r_d = nc.dram_tensor("w2r", [P, KB, D], BF16_T, kind="ExternalInput")
    w3l_d = nc.dram_tensor("w3l", [P, KB, DO], BF16_T, kind="ExternalInput")
    w3r_d = nc.dram_tensor("w3r", [P, KB, DO], BF16_T, kind="ExternalInput")
    bias_d = nc.dram_tensor("bias", [P, 2 * KB + 1], F32, kind="ExternalInput")
    out_d = nc.dram_tensor("out", [NPC, DO], F32, kind="ExternalOutput")

    rg = [list(range(n_cores))]

    with tile.TileContext(nc) as tc, ExitStack() as ctx:
        consts = ctx.enter_context(tc.tile_pool(name="consts", bufs=1))
        gpool = ctx.enter_context(tc.tile_pool(name="gpool", bufs=g_bufs))
        work = ctx.enter_context(tc.tile_pool(name="work", bufs=3))
        dram = ctx.enter_context(tc.tile_pool(name="dram", bufs=1, space="DRAM"))
        psA = ctx.enter_context(tc.tile_pool(name="psA", bufs=2, space="PSUM"))
        psB = ctx.enter_context(tc.tile_pool(name="psB", bufs=2, space="PSUM"))
        psT = ctx.enter_context(tc.tile_pool(name="psT", bufs=2, space="PSUM"))
        psP = ctx.enter_context(tc.tile_pool(name="psP", bufs=2, space="PSUM"))

        # ---- internal DRAM: collective bounce + gathered h (one Shared
        # tensor per AllGather chunk: Shared tensors are single-writer)
        r0, base1 = pl.AG_R0, pl.AG_BASE1
        cc1 = dram.tile([NPC, D], FP8_T, name="cc1")
        ccp = dram.tile([NPC, DO], BF16_T, name="ccp")
        h1_parts = [dram.tile([base1, D], FP8_T, addr_space="Shared",
                              name="h1a")]
        p_parts = [dram.tile([base1, DO], BF16_T, addr_space="Shared",
                             name="pa")]
        if pl.NPIECE == 2:
            h1_parts.append(dram.tile([NPAD - base1, D], FP8_T,
                                      addr_space="Shared", name="h1b"))
            p_parts.append(dram.tile([NPAD - base1, DO], BF16_T,
                                     addr_space="Shared", name="pb"))

        # ---- resident SBUF. Only what group 0 needs is loaded up front;
        # everything else (idx, weights for later layers, 1/deg, hT
        # blocks, S slices) is interleaved into the early groups so the
        # first aggregation matmuls start within a few microseconds.
        s_sb = consts.tile([P, NCHUNK, P], FP8_T)
        idx_sb = consts.tile([P, NCHUNK * 8], I16)
        idg_sb = consts.tile([P, NB, P], BF16_T)
        w1l = consts.tile([P, KB, D], BF16_T)
        nc.sync.dma_start(w1l[:], w1l_d[:])
        w1r = consts.tile([P, KB, D], BF16_T)
        nc.sync.dma_start(w1r[:], w1r_d[:])
        bias_sb = consts.tile([P, 2 * KB + 1], F32)
        nc.sync.dma_start(bias_sb[:], bias_d[:])
        w2l = consts.tile([P, KB, D], BF16_T)
        w2r = consts.tile([P, KB, D], BF16_T)
        w3l = consts.tile([P, KB, DO], BF16_T)
        w3r = consts.tile([P, KB, DO], BF16_T)
        ident_b = consts.tile([P, P], BF16_T)
        make_identity(nc, ident_b)
        ident_f = consts.tile([P, P], F32)
        make_identity(nc, ident_f)
        deferred = [
            (idg_sb, idg_d), (idx_sb, idx_d),
            (w2l, w2l_d), (w2r, w2r_d), (w3l, w3l_d), (w3r, w3r_d),
        ]

        # per-block resident hT tiles (feature-major current features,
        # loaded lazily in their supergroup's first-piece load slot) and
        # lo-piece partial segment sums
        ht = []
        m_lo = []
        for b in range(NB):
            t = consts.tile([P, KB, P], BF16_T, name=f"ht{b}", tag=f"ht{b}")
            ht.append(t)
            m_lo.append(consts.tile([P, KB, P], BF16_T, name=f"ml{b}",
                                    tag=f"ml{b}"))

        layers = [
            dict(wl=w1l, wr=w1r, mask=m1_d, bcol=0,
                 stream=pg1_d, src=None, elem=D),
            dict(wl=w2l, wr=w2r, mask=m2_d, bcol=KB,
                 src=[h1_parts[q][:, :] for q in range(NQ)], elem=D),
            dict(wr3=w3r, bcol=2 * KB,
                 src=[p_parts[q][:, :] for q in range(NQ)], elem=DO),
        ]

        max_bpg = max(len(sg["blocks"]) for sg in pl.sgs)
        gq = [0]
        half_cap = (GMAX + 1) // 2
        NSG = len(pl.sgs)
        LOOK = max(1, look) + 1

        # ---- two-pass layer schedule: each layer runs all supergroups'
        # lo-piece gathers + partial segment sums (into m_lo, SBUF) first,
        # then the hi pass re-adds the partial via an identity matmul and
        # finishes the block (dense, activation, AllGather chunks). The
        # gather stream therefore never stalls on AllGather chunk b, which
        # is hidden behind the whole lo pass of the next layer.
        groups = [(li, q, si)
                  for li in range(3) for q in range(NQ) for si in range(NSG)]

        def emit_group_loads(li, q, si):
            L = layers[li]
            sg = pl.sgs[si]
            last3 = li == 2
            st = dict(tiles={}, n1=0, mks={})
            n = sg["q_n"][q]
            s0 = sg["q_start"][q]
            if li == 0:
                # interleave the bulk constant loads into early layer-1
                # groups: this group's S slice, its blocks' hT, and one
                # deferred global per group
                if n > 0:
                    nc.sync.dma_start(s_sb[:, s0:s0 + n, :],
                                      s01_d[:, s0:s0 + n, :])
                if q == 0:
                    for b in sg["blocks"]:
                        nc.sync.dma_start(
                            ht[b][:], xT_d[:, :, b * P:(b + 1) * P])
                if deferred:
                    t, d = deferred.pop(0)
                    nc.sync.dma_start(t[:], d[:])
            else:
                while deferred:
                    t, d = deferred.pop(0)
                    nc.sync.dma_start(t[:], d[:])
            if n > 0:
                n1 = (n + 1) // 2
                st["n1"] = n1
                for sub, (o, m) in enumerate(((0, n1), (n1, n - n1))):
                    if m == 0:
                        continue
                    g_t = gpool.tile([P, half_cap, D], FP8_T,
                                     tag=f"g{q}_{sub}", bufs=LOOK + 1)
                    if last3:
                        gv = g_t[:].bitcast(BF16_T)
                    else:
                        gv = g_t
                    if L.get("stream") is not None:
                        # layer 1: pre-gathered on the host — contiguous
                        # stream on the (otherwise idle in layer 1) SWDGE
                        # queue, keeping the sync HWDGE queue for masks/hT
                        nc.gpsimd.dma_start(
                            g_t[:, :m, :],
                            L["stream"][:, s0 + o:s0 + o + m, :])
                    else:
                        nc.gpsimd.dma_gather(
                            gv[:, :m, :L["elem"]], L["src"][q],
                            idx_sb[:, (s0 + o) * 8:(s0 + o + m) * 8],
                            m * P, m * P, L["elem"], single_packet=False,
                            queue_num=gq[0] % 4)
                        gq[0] += 1
                    st["tiles"][sub] = gv
            if q == NQ - 1 and not last3:
                for b in sg["blocks"]:
                    mk_t = work.tile([P, KB, P], BF16_T, tag="mk",
                                     bufs=max_bpg + 4)
                    nc.sync.dma_start(
                        mk_t[:], L["mask"][:, :, b * P:(b + 1) * P])
                    st["mks"][b] = mk_t
            return st

        def emit_group_compute(li, q, si, st):
            L = layers[li]
            sg = pl.sgs[si]
            last3 = li == 2
            last_pass = q == NQ - 1
            tiles, n1, mks = st["tiles"], st["n1"], st["mks"]
            for b in sg["blocks"]:
                my = []
                for (s, qq, sl, cg) in pl.block_chunks[b]:
                    if s != si or qq != q:
                        continue
                    if sl < n1:
                        my.append((tiles[0], sl, cg))
                    else:
                        my.append((tiles[1], sl - n1, cg))
                nch = len(my)
                has_lo = NQ == 2 and pl.nchunk_bq[b][0] > 0

                if not last_pass:
                    # lo pass: partial segment sum -> m_lo[b]
                    if nch == 0:
                        continue
                    mps = psA.tile([P, KB, P], F32, tag="acc")
                    nk = 1 if last3 else KB
                    for k in range(nk):
                        for ci, (gt, sl, cg) in enumerate(my):
                            nc.tensor.matmul(
                                mps[:, k, :],
                                gt[:, sl, k * P:(k + 1) * P],
                                s_sb[:, cg, :],
                                start=(ci == 0), stop=(ci == nch - 1))
                    nc.vector.tensor_copy(m_lo[b][:, :nk, :],
                                          mps[:, :nk, :])
                    continue

                bsl = slice(b * P, (b + 1) * P)
                if not last3:
                    # segment mean (transposed): meanT[feat,dst], re-adding
                    # the lo-pass partial via an identity matmul
                    mps = psA.tile([P, KB, P], F32, tag="acc")
                    nops = nch + (1 if has_lo else 0)
                    for k in range(KB):
                        for ci, (gt, sl, cg) in enumerate(my):
                            nc.tensor.matmul(
                                mps[:, k, :],
                                gt[:, sl, k * P:(k + 1) * P],
                                s_sb[:, cg, :],
                                start=(ci == 0), stop=(ci == nops - 1))
                        if has_lo:
                            nc.tensor.matmul(
                                mps[:, k, :], ident_b, m_lo[b][:, k, :],
                                start=(nch == 0), stop=True)
                    m_sb = work.tile([P, KB, P], BF16_T, tag="msb")
                    if nch == 0 and not has_lo:
                        nc.vector.memset(m_sb[:], 0.0)
                    else:
                        # fold the 1/deg mean scaling into the
                        # PSUM->SBUF copy (S is a 0/1 one-hot)
                        i0, i1 = bass.broadcast_tensor_aps(
                            mps[:], idg_sb[:, b:b + 1, :])
                        nc.vector.tensor_tensor(
                            out=m_sb[:], in0=i0, in1=i1,
                            op=mybir.AluOpType.mult)

                    # dense: outT[feat_out, node] = Wl@meanT + Wr@hT
                    ops = psB.tile([P, KB, P], F32, tag="out")
                    for bank in range(KB):
                        for k in range(KB):
                            nc.tensor.matmul(
                                ops[:, bank, :],
                                L["wl"][:, k, bank * P:(bank + 1) * P],
                                m_sb[:, k, :],
                                start=(k == 0), stop=False)
                        for k in range(KB):
                            nc.tensor.matmul(
                                ops[:, bank, :],
                                L["wr"][:, k, bank * P:(bank + 1) * P],
                                ht[b][:, k, :],
                                start=False, stop=(k == KB - 1))

                    # epilogue: tanh(+bias), dropout mask, update hT
                    mk_t = mks[b]
                    a_sb = work.tile([P, KB, P], BF16_T, tag="act")
                    for bank in range(KB):
                        nc.scalar.activation(
                            a_sb[:, bank, :], ops[:, bank, :], Tanh,
                            bias=bias_sb[:, L["bcol"] + bank:
                                         L["bcol"] + bank + 1])
                    nc.vector.tensor_mul(
                        out=ht[b][:], in0=a_sb[:], in1=mk_t[:])

                    row0 = b * P
                    if li == 0:
                        # node-major copy for AllGather input
                        nm = work.tile([P, D], FP8_T, tag="nm")
                        for bank in range(KB):
                            tp = psT.tile([P, P], BF16_T, tag="tp")
                            nc.tensor.transpose(
                                tp, ht[b][:, bank, :], ident_b)
                            nc.vector.tensor_copy(
                                nm[:, bank * P:(bank + 1) * P], tp)
                        nc.sync.dma_start(cc1[row0:row0 + P, :], nm)

                    if li == 1:
                        # p = h2 @ W3l^T (node-major) for layer-3 gather
                        pp = psP.tile([P, DO], F32, tag="pp")
                        for k in range(KB):
                            nc.tensor.matmul(
                                pp, ht[b][:, k, :], w3l[:, k, :],
                                start=(k == 0), stop=(k == KB - 1))
                        p_sb = work.tile([P, DO], BF16_T, tag="pnm")
                        nc.vector.tensor_copy(p_sb, pp)
                        nc.sync.dma_start(ccp[row0:row0 + P, :], p_sb)

                    # layer boundary AllGather, split in two chunks so
                    # chunk a overlaps the tail of this layer's compute
                    # and chunk b hides behind the next layer's lo pass
                    cin = cc1 if li == 0 else ccp
                    cparts = h1_parts if li == 0 else p_parts
                    if b == pl.S1B - 1:
                        nc.gpsimd.collective_compute(
                            "AllGather", mybir.AluOpType.bypass,
                            replica_groups=rg,
                            ins=[cin[0:r0, :]], outs=[cparts[0].opt()])
                    if b == NB - 1 and pl.NPIECE == 2:
                        nc.gpsimd.collective_compute(
                            "AllGather", mybir.AluOpType.bypass,
                            replica_groups=rg,
                            ins=[cin[r0:NPC, :]], outs=[cparts[1].opt()])
                else:
                    # layer 3: outT = mean(p)^T + W3r @ hT, tanh, output
                    # (segment sum in its own accumulator so the 1/deg
                    # scaling can apply before joining the dense term)
                    any_agg = nch > 0 or has_lo
                    if any_agg:
                        mps3 = psA.tile([P, KB, P], F32, tag="acc")
                        m3 = mps3[:, 0, :]
                        nops = nch + (1 if has_lo else 0)
                        for ci, (gt, sl, cg) in enumerate(my):
                            nc.tensor.matmul(
                                m3, gt[:, sl, :], s_sb[:, cg, :],
                                start=(ci == 0), stop=(ci == nops - 1))
                        if has_lo:
                            nc.tensor.matmul(
                                m3, ident_b, m_lo[b][:, 0, :],
                                start=(nch == 0), stop=True)
                        m3_sb = work.tile([P, P], BF16_T, tag="m3")
                        nc.vector.tensor_tensor(
                            out=m3_sb, in0=m3, in1=idg_sb[:, b, :],
                            op=mybir.AluOpType.mult)
                    ops = psB.tile([P, KB, P], F32, tag="out")
                    o3 = ops[:, 0, :]
                    for k in range(KB):
                        nc.tensor.matmul(
                            o3, L["wr3"][:, k, :], ht[b][:, k, :],
                            start=(k == 0),
                            stop=(not any_agg and k == KB - 1))
                    if any_agg:
                        nc.tensor.matmul(
                            o3, ident_b, m3_sb, start=False, stop=True)
                    o_sb = work.tile([P, DO], F32, tag="o3")
                    nc.scalar.activation(
                        o_sb, o3, Tanh,
                        bias=bias_sb[:, L["bcol"]:L["bcol"] + 1])
                    tpf = psP.tile([P, DO], F32, tag="pp")
                    nc.tensor.transpose(tpf, o_sb, ident_f)
                    onm = work.tile([P, DO], F32, tag="onm")
                    nc.vector.tensor_copy(onm, tpf)
                    nc.sync.dma_start(out_d[bsl, :], onm)

        # a group's gathers may only be emitted after the compute group
        # that launches the AllGather chunk they read from (otherwise the
        # read has no writer yet and Tile cannot order them)
        si_ag = next(si for si, sg in enumerate(pl.sgs)
                     if pl.S1B - 1 in sg["blocks"])
        gidx = {g: i for i, g in enumerate(groups)}
        req = []
        for (li, q, si) in groups:
            if li == 0:
                req.append(-1)
            else:
                tgt = NSG - 1 if (q == NQ - 1 or NQ == 1) else si_ag
                req.append(gidx[(li - 1, NQ - 1, tgt)])

        states = {}
        load_cursor = 0
        for gi in range(len(groups)):
            target = min(gi + LOOK, len(groups) - 1)
            while load_cursor <= target and req[load_cursor] <= gi - 1:
                states[load_cursor] = emit_group_loads(*groups[load_cursor])
                load_cursor += 1
            emit_group_compute(*groups[gi], states.pop(gi))

    nc.compile()
    return nc


# ----------------------------------------------------------------------------
# Host driver
# ----------------------------------------------------------------------------

def prepare(x, edge_index, mask1, mask2,
            W1l, b1, W1r, W2l, b2, W2r, W3l, b3, W3r, gmax=12):
    N, D = x.shape
    DO = W3l.shape[0]
    E = edge_index.shape[1]
    pl = make_plan(N, E, D, DO, edge_index, gmax=gmax)
    KB = pl.KB

    x_bf = x.astype(BF16)
    scale1 = ((mask1 > DROP_P) / (1.0 - DROP_P)).astype(np.float32)
    scale2 = ((mask2 > DROP_P) / (1.0 - DROP_P)).astype(np.float32)

    def packw(W):
        return np.ascontiguousarray(
            W.T.reshape(KB, P, W.shape[0]).transpose(1, 0, 2)).astype(BF16)

    bias = np.zeros((P, 2 * KB + 1), dtype=np.float32)
    for k in range(KB):
        bias[:, k] = b1[k * P:(k + 1) * P]
        bias[:, KB + k] = b2[k * P:(k + 1) * P]
    bias[:, 2 * KB] = b3[:P]

    shared = {
        "w1l": packw(W1l), "w1r": packw(W1r),
        "w2l": packw(W2l), "w2r": packw(W2r),
        "w3l": packw(W3l), "w3r": packw(W3r),
        "bias": bias,
    }
    in_maps = []
    for c in range(NCORES):
        m = dict(shared)
        m.update(pack_core(pl, c, x_bf, scale1, scale2))
        in_maps.append(m)
    return pl, in_maps


def kernel(x, edge_index, mask1, mask2,
           W1l, b1, W1r, W2l, b2, W2r, W3l, b3, W3r):
    x = np.asarray(x, dtype=np.float32)
    pl, in_maps = prepare(
        x, np.asarray(edge_index),
        np.asarray(mask1, dtype=np.float32),
        np.asarray(mask2, dtype=np.float32),
        np.asarray(W1l, np.float32), np.asarray(b1, np.float32),
        np.asarray(W1r, np.float32),
        np.asarray(W2l, np.float32), np.asarray(b2, np.float32),
        np.asarray(W2r, np.float32),
        np.asarray(W3l, np.float32), np.asarray(b3, np.float32),
        np.asarray(W3r, np.float32))
    nc = build_program(pl)
    res = run_bass_kernel_spmd(nc, in_maps, core_ids=list(range(NCORES)))
    N = x.shape[0]
    out = np.zeros((N, pl.DO), dtype=np.float32)
    for c in range(NCORES):
        lo, hi = c * pl.NPC, min((c + 1) * pl.NPC, N)
        out[lo:hi] = res.results[c]["out"][:hi - lo]
    return out



# revision 35
# speedup vs baseline: 1.0605x; 1.0605x over previous
"""GraphSAGE 3-layer GNN on 8 TRN2 NeuronCores.

Strategy (node-sharded, feature-replicated):
  - 50000 nodes padded to 50176 = 8 cores x 49 blocks x 128. Core c owns
    destination rows [6272c, 6272c+6272).
  - Per layer, every core gathers h[src] rows for its local edges with
    gpsimd.dma_gather (one 512B/256B row per edge, landing on one SBUF
    partition), then segment-sums them per 128-dst block with a one-hot
    (x 1/deg) matmul on TensorE:  meanT[feat,dst] += G[e,feat].T @ S[e,dst].
    S (one-hot x 1/deg) and the int16 gather indices are host-precomputed
    and SBUF-resident; the node space is split at HALF (host-optimized)
    so indices fit int16, giving a lo and a hi gather per supergroup,
    each further split in two and spread over the 4 SWDGE queues so all
    4 Q7 core pairs generate DMA descriptors concurrently (the gather
    stream, ~3.3 ns/edge, is the kernel's critical path).
  - Dense SAGE terms run in transposed orientation so tanh bias is
    per-partition:  outT[feat_out, node] = Wl @ meanT + Wr @ hT (+bias);
    hT (feature-major current features) stays SBUF-resident per block.
  - Dropout masks are precomputed on the host as {0, 1.25} multipliers.
  - One AllGather per layer boundary replicates the node-major output
    (via PE transpose) for the next layer's gathers. Layer 3 aggregates
    the W3l-projected features p = h2 @ W3l^T (128 dims), so only p is
    gathered; h2 stays local for the W3r term.
  - Emission is software-pipelined two-cursor: gathers/masks for
    supergroup k+LOOK are emitted before compute of supergroup k so the
    in-order engine queues never serialize consecutive supergroups.

kernel(**inputs) takes full-size numpy inputs, does all index/layout prep on
the host, compiles one SPMD Bass program, runs it on cores 0-7, and returns
the full [50000, 128] float32 output.
"""

import math
from contextlib import ExitStack

import ml_dtypes
import numpy as np

import bass_rust
import concourse.bass as bass
import concourse.bass_isa as bass_isa
import concourse.bacc as bacc
import concourse.mybir as mybir
import concourse.tile as tile
from concourse.bass_utils import run_bass_kernel_spmd
from concourse.masks import make_identity

P = 128
NCORES = 8
DROP_P = 0.2
BF16 = ml_dtypes.bfloat16

F32 = mybir.dt.float32
BF16_T = mybir.dt.bfloat16
I16 = mybir.dt.int16
FP8_T = mybir.dt.float8e4
FP8 = ml_dtypes.float8_e4m3


class Plan:
    pass


# ----------------------------------------------------------------------------
# Planning (shared across cores; the SPMD program structure depends on it)
# ----------------------------------------------------------------------------

def make_plan(n_nodes, n_edges, d_in, d_out, edge_index, gmax=24):
    pl = Plan()
    pl.N = n_nodes
    pl.E = n_edges
    pl.D = d_in
    pl.DO = d_out
    pl.KB = d_in // P
    assert d_in % P == 0 and d_out == P
    pl.NB = math.ceil(n_nodes / NCORES / P)     # blocks per core
    pl.NPC = pl.NB * P                          # padded nodes per core
    pl.NPAD = NCORES * pl.NPC

    src = np.asarray(edge_index[0], dtype=np.int64)
    dst = np.asarray(edge_index[1], dtype=np.int64)
    assert src.min() >= 0 and src.max() < n_nodes
    deg = np.bincount(dst, minlength=n_nodes)
    pl.invdeg = (1.0 / np.maximum(deg, 1)).astype(np.float32)

    core = dst // pl.NPC
    lb = (dst - core * pl.NPC) // P             # dst block within core
    pl.dst_local = (dst - core * pl.NPC) % P
    pl.core = core
    pl.lb = lb
    pl.src = src
    pl.dst = dst

    # Shared h tables use an AllGather-chunk-major layout: each layer's
    # AllGather is split in two (blocks [0,S1B) then [S1B,NB)) so the
    # first chunk's transfer overlaps the tail of the layer's compute,
    # and the second is hidden behind the next layer's lo-piece gathers.
    # The lo/hi gather-piece boundary coincides with the chunk boundary,
    # so each piece gathers from its own (single-writer) Shared tensor.
    # Node (core c, local i) sits at position:
    #   i <  r0:  c*r0 + i                       (chunk a, piece lo)
    #   i >= r0:  8*r0 + c*r1 + (i - r0)         (chunk b, piece hi)
    # S1B must keep both pieces' index windows inside int16 and is chosen
    # to minimize total padded chunks.
    if pl.NPAD <= 32768:
        npiece = 1
        pl.S1B = pl.NB
        best = None
    else:
        npiece = 2
        loc = src % pl.NPC
        s_lo = -(-(pl.NPAD - 32768) // (NCORES * P))     # piece-hi window
        s_hi = 32768 // (NCORES * P)                     # piece-lo window
        assert s_lo <= s_hi and s_hi >= 1
        best = None
        key_base = (core * pl.NB + lb) * 2
        for s1b in range(s_lo, min(s_lo + 3, s_hi, pl.NB - 1) + 1):
            is_hi = (loc >= s1b * P).astype(np.int64)
            cnt = np.bincount(key_base + is_hi,
                              minlength=NCORES * pl.NB * 2)
            tot = int((-(-cnt.reshape(NCORES, pl.NB, 2).max(axis=0)
                         // P)).sum())
            if best is None or tot < best[0]:
                best = (tot, s1b)
        pl.S1B = best[1]
    r0 = pl.S1B * P
    r1 = pl.NPC - r0
    pl.AG_R0 = r0
    pl.AG_BASE1 = NCORES * r0
    nodes = np.arange(n_nodes)
    nc_, ni = nodes // pl.NPC, nodes % pl.NPC
    pl.pad_of_node = np.where(
        ni < r0, nc_ * r0 + ni, pl.AG_BASE1 + nc_ * r1 + (ni - r0))
    srcpos = pl.pad_of_node[src]
    pl.HALF = pl.AG_BASE1 if npiece == 2 else pl.NPAD
    pl.src_q = (srcpos >= pl.HALF).astype(np.int64)
    pl.src_idx = np.where(pl.src_q == 1, srcpos - pl.HALF, srcpos)
    pl.NPIECE = npiece
    pl.gpb = np.array([0, pl.HALF, pl.NPAD][:npiece + 1], dtype=np.int64)
    assert pl.src_idx.max() < 32768

    # chunk caps per (dst block, src piece): max need over cores
    key = (core * pl.NB + lb) * npiece + pl.src_q
    cnt = np.bincount(key, minlength=NCORES * pl.NB * npiece)
    cnt = cnt.reshape(NCORES, pl.NB, npiece)
    pl.nchunk_bq = -(-cnt.max(axis=0) // P)    # [NB, npiece]

    # supergroups: consecutive blocks, per-piece chunk sums <= gmax
    sgs = []
    cur, s_q = [], np.zeros(npiece, dtype=np.int64)
    for b in range(pl.NB):
        if cur and (s_q + pl.nchunk_bq[b] > gmax).any():
            sgs.append(cur)
            cur, s_q = [], np.zeros(npiece, dtype=np.int64)
        cur.append(b)
        s_q = s_q + pl.nchunk_bq[b]
    if cur:
        sgs.append(cur)

    # global chunk ordering: per sg -> per piece -> blocks in order
    pl.sgs = []
    cg = 0
    pl.block_chunks = [[] for _ in range(pl.NB)]  # (sg, piece, slot, cg)
    for si, blocks in enumerate(sgs):
        info = {"blocks": blocks, "q_start": [], "q_n": []}
        for q in range(npiece):
            start = cg
            for b in blocks:
                for _ in range(pl.nchunk_bq[b][q]):
                    pl.block_chunks[b].append((si, q, cg - start, cg))
                    cg += 1
            info["q_start"].append(start)
            info["q_n"].append(cg - start)
        pl.sgs.append(info)
    pl.NCHUNK = cg
    pl.GMAX = max(max(sg["q_n"]) for sg in pl.sgs)
    pl.SMAX = max(sum(sg["q_n"]) for sg in pl.sgs)
    return pl


# ----------------------------------------------------------------------------
# Per-core host packing
# ----------------------------------------------------------------------------

def _featT(arr, KB):
    """[n, KB*128] -> [128, KB, n]  (feature-major layout)."""
    n = arr.shape[0]
    return np.ascontiguousarray(arr.T.reshape(KB, P, n).transpose(1, 0, 2))


def pack_core(pl, c, x_padded, scale1, scale2):
    NB, NPC, NCHUNK, NQ = pl.NB, pl.NPC, pl.NCHUNK, pl.NPIECE
    mine = pl.core == c
    sidx, q, lb, dl = (pl.src_idx[mine], pl.src_q[mine],
                       pl.lb[mine], pl.dst_local[mine])
    w = pl.invdeg[pl.dst[mine]]

    # sort by src within each (block, piece) group: gathered rows then read
    # the shared h table in ascending address order (DRAM page locality)
    order = np.lexsort((sidx, q, lb))
    sidx, q, lb, dl, w = (a[order] for a in (sidx, q, lb, dl, w))

    # chunk start (cg) per (block, piece)
    cg_start = np.zeros((NB, NQ), dtype=np.int64)
    for b in range(NB):
        for (si, qq, sl, cg) in pl.block_chunks[b]:
            if cg_start[b, qq] == 0 or sl == 0:
                pass
        firsts = {}
        for (si, qq, sl, cg) in pl.block_chunks[b]:
            if qq not in firsts:
                firsts[qq] = cg
        for qq, cg0 in firsts.items():
            cg_start[b, qq] = cg0

    key = lb * NQ + q
    grp_first = np.searchsorted(key, np.arange(NB * NQ))
    rank = np.arange(len(sidx)) - grp_first[key]
    assert (rank < pl.nchunk_bq[lb, q] * P).all(), "chunk overflow"
    pos = cg_start[lb, q] * P + rank

    idx_vals = np.zeros(NCHUNK * P, dtype=np.int16)
    idx_vals[pos] = sidx.astype(np.int16)

    # trailing pad slots of each (supergroup, piece) gather window become -1:
    # the SWDGE ucode trims trailing negative indices before descriptor
    # emission, so per-core trailing padding costs neither Q7 time nor DMA
    # bandwidth (interior pads stay 0 -- negative interior indices would be
    # used as addresses). The consuming one-hot S columns are zero either way.
    has_real = np.zeros(NCHUNK * P, dtype=bool)
    has_real[pos] = True
    for sg in pl.sgs:
        for qq in range(NQ):
            s0, n = sg["q_start"][qq], sg["q_n"][qq]
            if n == 0:
                continue
            w0, w1 = s0 * P, (s0 + n) * P
            real = np.flatnonzero(has_real[w0:w1])
            last = (w0 + real[-1]) if real.size else (w0 - 1)
            idx_vals[last + 1:w1] = 0  # trim disabled: sim asserts reg==valid count; stale-byte reads flagged

    # 0/1 one-hot scatter matrix in fp8 (exact); the 1/deg mean scaling is
    # applied post-sum from invdegT, so S stays 8.9MB instead of 17.7MB
    S01 = np.zeros((P, NCHUNK, P), dtype=FP8)
    S01[pos % P, pos // P, dl] = 1.0

    # per-destination 1/deg, feature-broadcast: invdegT[p, b, d] for the
    # core's dst node (b, d), identical on every partition p
    idg = np.ones((NB * P,), dtype=np.float32)
    lo_node = c * NPC
    n_real = min(NPC, pl.N - lo_node)
    idg[:n_real] = pl.invdeg[lo_node:lo_node + n_real]
    invdegT = np.broadcast_to(
        idg.reshape(1, NB, P), (P, NB, P)).astype(BF16)

    # layer-1 pre-gathered edge features (host-side index copy of x): slot
    # pos holds x[src] so layer 1 streams contiguous DMAs instead of
    # per-row SWDGE gathers
    srcg = pl.src[mine][order]
    pg1 = np.zeros((P, NCHUNK, pl.D), dtype=FP8)
    pg1[pos % P, pos // P, :] = x_padded[srcg].astype(FP8)

    # wrap idx into [128, NCHUNK*8] int16, per-gather-call 16-element wrap
    idx_all = np.zeros((P, NCHUNK * 8), dtype=np.int16)
    for sg in pl.sgs:
        for qq in range(NQ):
            s0, n = sg["q_start"][qq], sg["q_n"][qq]
            if n == 0:
                continue
            seg = idx_vals[s0 * P:(s0 + n) * P]
            blk = seg.reshape(-1, 16).T
            idx_all[:, s0 * 8:(s0 + n) * 8] = np.tile(blk, (8, 1))

    lo_node, hi_node = c * NPC, min((c + 1) * NPC, pl.N)
    nreal = hi_node - lo_node
    xo = np.zeros((NPC, pl.D), dtype=BF16)
    xo[:nreal] = x_padded[lo_node:hi_node]
    m1 = np.zeros((NPC, pl.D), dtype=np.float32)
    m2 = np.zeros((NPC, pl.D), dtype=np.float32)
    m1[:nreal] = scale1[lo_node:hi_node]
    m2[:nreal] = scale2[lo_node:hi_node]

    return {
        "s01": S01,
        "invdegT": np.ascontiguousarray(invdegT),
        "idx_all": idx_all,
        "pg1": pg1,
        "xT": _featT(xo, pl.KB).astype(BF16),
        "m1T": _featT(m1, pl.KB).astype(FP8),
        "m2T": _featT(m2, pl.KB).astype(FP8),
    }


# ----------------------------------------------------------------------------
# Bass program
# ----------------------------------------------------------------------------

def build_program(pl, n_cores=NCORES, g_bufs=3, s_bufs=2, look=3, boost=1,
                  zero_bias=False, use_prep=False):
    # Keep prepared gathers off Tile's DMASW semaphore lanes for the duration
    # of this build: the hardware descriptor encodes exactly one completion
    # semaphore (the sem= one), so a DMASW lane sem attached on top would
    # never increment on HW and the end-of-program drain would hang waiting
    # for it. Consumer ordering is handled by explicit wait_ge on our sems.
    orig_usrd = bass_isa.UserSyncedRemoteDMADescs
    if use_prep:
        bass_isa.UserSyncedRemoteDMADescs = orig_usrd | mybir.InstDMAGatherAnt
    try:
        return _build_program(pl, n_cores, g_bufs, s_bufs, look, boost,
                              zero_bias, use_prep)
    finally:
        bass_isa.UserSyncedRemoteDMADescs = orig_usrd


def _build_program(pl, n_cores, g_bufs, s_bufs, look, boost, zero_bias,
                   use_prep):
    D, DO, KB, NB, NPC, NPAD = pl.D, pl.DO, pl.KB, pl.NB, pl.NPC, pl.NPAD
    NCHUNK, GMAX, SMAX, NQ = pl.NCHUNK, pl.GMAX, pl.SMAX, pl.NPIECE
    Tanh = mybir.ActivationFunctionType.Tanh

    nc = bacc.Bacc("TRN2", target_bir_lowering=False, debug=False,
                   num_devices=n_cores, dynamic_dma_scratch_size=8192,
                   num_swdge_queues=4)

    # ---- DRAM parameters
    pg1_d = nc.dram_tensor("pg1", [P, NCHUNK, D], FP8_T, kind="ExternalInput")
    xT_d = nc.dram_tensor("xT", [P, KB, NPC], BF16_T, kind="ExternalInput")
    m1_d = nc.dram_tensor("m1T", [P, KB, NPC], FP8_T, kind="ExternalInput")
    m2_d = nc.dram_tensor("m2T", [P, KB, NPC], FP8_T, kind="ExternalInput")
    s01_d = nc.dram_tensor("s01", [P, NCHUNK, P], FP8_T, kind="ExternalInput")
    idg_d = nc.dram_tensor("invdegT", [P, NB, P], BF16_T, kind="ExternalInput")
    idx_d = nc.dram_tensor("idx_all", [P, NCHUNK * 8], I16, kind="ExternalInput")
    w1l_d = nc.dram_tensor("w1l", [P, KB, D], BF16_T, kind="ExternalInput")
    w1r_d = nc.dram_tensor("w1r", [P, KB, D], BF16_T, kind="ExternalInput")
    w2l_d = nc.dram_tensor("w2l", [P, KB, D], BF16_T, kind="ExternalInput")
    w2r_d = nc.dram_tensor("w2r", [P, KB, D], BF16_T, kind="ExternalInput")
    w3l_d = nc.dram_tensor("w3l", [P, KB, DO], BF16_T, kind="ExternalInput")
    w3r_d = nc.dram_tensor("w3r", [P, KB, DO], BF16_T, kind="ExternalInput")
    bias_d = nc.dram_tensor("bias", [P, 2 * KB + 1], F32, kind="ExternalInput")
    out_d = nc.dram_tensor("out", [NPC, DO], F32, kind="ExternalOutput")

    rg = [list(range(n_cores))]

    with tile.TileContext(nc) as tc, ExitStack() as ctx:
        consts = ctx.enter_context(tc.tile_pool(name="consts", bufs=1))
        gpool = ctx.enter_context(tc.tile_pool(name="gpool", bufs=g_bufs))
        work = ctx.enter_context(tc.tile_pool(name="work", bufs=3))
        dram = ctx.enter_context(tc.tile_pool(name="dram", bufs=1, space="DRAM"))
        psA = ctx.enter_context(tc.tile_pool(name="psA", bufs=2, space="PSUM"))
        psB = ctx.enter_context(tc.tile_pool(name="psB", bufs=2, space="PSUM"))
        psT = ctx.enter_context(tc.tile_pool(name="psT", bufs=2, space="PSUM"))
        psP = ctx.enter_context(tc.tile_pool(name="psP", bufs=2, space="PSUM"))

        # ---- internal DRAM: collective bounce + gathered h (one Shared
        # tensor per AllGather chunk: Shared tensors are single-writer)
        r0, base1 = pl.AG_R0, pl.AG_BASE1
        cc1 = dram.tile([NPC, D], FP8_T, name="cc1")
        ccp = dram.tile([NPC, DO], BF16_T, name="ccp")
        h1_parts = [dram.tile([base1, D], FP8_T, addr_space="Shared",
                              name="h1a")]
        p_parts = [dram.tile([base1, DO], BF16_T, addr_space="Shared",
                             name="pa")]
        if pl.NPIECE == 2:
            h1_parts.append(dram.tile([NPAD - base1, D], FP8_T,
                                      addr_space="Shared", name="h1b"))
            p_parts.append(dram.tile([NPAD - base1, DO], BF16_T,
                                     addr_space="Shared", name="pb"))

        # ---- resident SBUF. Only what group 0 needs is loaded up front;
        # everything else (idx, weights for later layers, 1/deg, hT
        # blocks, S slices) is interleaved into the early groups so the
        # first aggregation matmuls start within a few microseconds.
        s_sb = consts.tile([P, NCHUNK, P], FP8_T)
        idx_sb = consts.tile([P, NCHUNK * 8], I16)
        idg_sb = consts.tile([P, NB, P], BF16_T)
        w1l = consts.tile([P, KB, D], BF16_T)
        nc.sync.dma_start(w1l[:], w1l_d[:])
        w1r = consts.tile([P, KB, D], BF16_T)
        nc.sync.dma_start(w1r[:], w1r_d[:])
        bias_sb = consts.tile([P, 2 * KB + 1], F32)
        nc.sync.dma_start(bias_sb[:], bias_d[:])
        w2l = consts.tile([P, KB, D], BF16_T)
        w2r = consts.tile([P, KB, D], BF16_T)
        w3l = consts.tile([P, KB, DO], BF16_T)
        w3r = consts.tile([P, KB, DO], BF16_T)
        ident_b = consts.tile([P, P], BF16_T)
        make_identity(nc, ident_b)
        ident_f = consts.tile([P, P], F32)
        make_identity(nc, ident_f)
        deferred = [
            (idg_sb, idg_d), (idx_sb, idx_d),
            (w2l, w2l_d), (w2r, w2r_d), (w3l, w3l_d), (w3r, w3r_d),
        ]

        # resident hT (feature-major current features, one tile, sliced per
        # block; hazards are range-tracked so per-block slices stay
        # independent) and lo-piece partial segment sums
        ht_all = consts.tile([P, KB, NB * P], BF16_T, name="ht_all")
        nc.sync.dma_start(ht_all[:], xT_d[:])
        ml_all = consts.tile([P, KB, NB * P], BF16_T, name="ml_all")
        ht = [ht_all[:, :, b * P:(b + 1) * P] for b in range(NB)]
        m_lo = [ml_all[:, :, b * P:(b + 1) * P] for b in range(NB)]

        # SWDGE prepare/trigger: descriptor emission on the Q7 (~3.8 ns/edge,
        # the kernel's true critical path) is decoupled from the data
        # dependency on the AllGather'd tables -- preps emit descriptors
        # early, the cheap trigger fires them once the table chunk has landed
        dma_sems = [nc.alloc_semaphore(f"swdge_dma{i}") for i in range(4)]
        prep_counts = [0, 0, 0, 0]
        ag_names = {}   # (consumer layer, piece) -> AllGather inst name

        # gather buffers see a read-modify-write at trigger time: zero them
        # ON THE GPSIMD ENGINE so the first write of these bytes is ordered
        # with the triggers by Pool program order (a cross-engine writer is
        # only no_sync-ordered against the deferred write)
        if use_prep:
            for qq in range(NQ):
                for _ in range(g_bufs):
                    t = gpool.tile([P, GMAX, D], FP8_T, tag=f"g{qq}",
                                   bufs=g_bufs)
                    nc.gpsimd.memset(t[:], 0.0)

        layers = [
            dict(wl=w1l, wr=w1r, mask=m1_d, bcol=0,
                 stream=pg1_d, src=None, elem=D),
            dict(wl=w2l, wr=w2r, mask=m2_d, bcol=KB,
                 src=[h1_parts[q][:, :] for q in range(NQ)], elem=D),
            dict(wr3=w3r, bcol=2 * KB,
                 src=[p_parts[q][:, :] for q in range(NQ)], elem=DO),
        ]

        max_bpg = max(len(sg["blocks"]) for sg in pl.sgs)
        gq = [0]
        NSG = len(pl.sgs)
        LOOK = max(1, look) + 1

        # ---- two-pass layer schedule: each layer runs all supergroups'
        # lo-piece gathers + partial segment sums (into m_lo, SBUF) first,
        # then the hi pass re-adds the partial via an identity matmul and
        # finishes the block (dense, activation, AllGather chunks). The
        # gather stream therefore never stalls on AllGather chunk b, which
        # is hidden behind the whole lo pass of the next layer.
        groups = [(li, q, si)
                  for li in range(3) for q in range(NQ) for si in range(NSG)]

        def emit_group_loads(li, q, si):
            L = layers[li]
            sg = pl.sgs[si]
            last3 = li == 2
            st = dict(tiles={}, mks=None)
            n = sg["q_n"][q]
            s0 = sg["q_start"][q]
            if li == 0:
                # interleave the bulk constant loads into early layer-1
                # groups: this group's S slice and one deferred global
                if n > 0:
                    nc.sync.dma_start(s_sb[:, s0:s0 + n, :],
                                      s01_d[:, s0:s0 + n, :])
                if deferred:
                    t, d = deferred.pop(0)
                    nc.sync.dma_start(t[:], d[:])
            else:
                while deferred:
                    t, d = deferred.pop(0)
                    nc.sync.dma_start(t[:], d[:])
            if n > 0:
                if L.get("stream") is not None:
                    # layer 1: pre-gathered on the host — contiguous stream
                    # on the sync HWDGE queue (GpSimd stays free so layer 2's
                    # descriptor preps can run during layer 1). Streams use
                    # their own buffer tags so gather preps never wait on
                    # layer-1 consumers.
                    g_t = gpool.tile([P, GMAX, D], FP8_T,
                                     tag=f"s{q}", bufs=s_bufs)
                    nc.sync.dma_start(
                        g_t[:, :n, :], L["stream"][:, s0:s0 + n, :])
                    st["tiles"][0] = g_t
                else:
                    # prepare-only gather: the Q7 emits descriptors now (the
                    # table read dep is deferred to the trigger), one call per
                    # (supergroup, piece) to amortize the ~1us fixed cost
                    g_t = gpool.tile([P, GMAX, D], FP8_T,
                                     tag=f"g{q}", bufs=g_bufs)
                    if last3:
                        gv = g_t[:].bitcast(BF16_T)
                    else:
                        gv = g_t
                    qn = gq[0] % 4
                    gq[0] += 1
                    if use_prep:
                        prep = nc.gpsimd.dma_gather(
                            gv[:, :n, :L["elem"]], L["src"][q],
                            idx_sb[:, s0 * 8:(s0 + n) * 8],
                            n * P, n * P, L["elem"], single_packet=False,
                            queue_num=qn, prepare_only=True,
                            sem=dma_sems[qn])
                        prep_counts[qn] += 1
                        st["trig_q"] = qn
                        st["prep"] = prep.ins
                        st["table_writer"] = ag_names.get((li, q))
                        # Tile does not attach the consumer-side wait for
                        # prepared gathers; emit an explicit PE-side wait_ge
                        st["dma_wait"] = (qn, 16 * prep_counts[qn])
                    else:
                        nc.gpsimd.dma_gather(
                            gv[:, :n, :L["elem"]], L["src"][q],
                            idx_sb[:, s0 * 8:(s0 + n) * 8],
                            n * P, n * P, L["elem"], single_packet=False,
                            queue_num=qn)
                    st["tiles"][0] = gv
            if q == NQ - 1 and not last3:
                nb = len(sg["blocks"])
                b0 = sg["blocks"][0]
                mk_t = work.tile([P, KB, max_bpg * P], FP8_T, tag="mk",
                                 bufs=LOOK + 2)
                nc.sync.dma_start(
                    mk_t[:, :, :nb * P],
                    L["mask"][:, :, b0 * P:(b0 + nb) * P])
                st["mks"] = (mk_t, b0)
            return st

        def emit_group_trigger(st):
            if "trig_q" not in st:
                return
            # Tile does not defer a gather prep's table-read dep to the
            # trigger (swdge_deferred_ins covers other inst types), so the
            # prep would stall descriptor emission until the AllGather
            # lands. Move the dep by hand: the prep keeps a no-sync edge
            # (sim topological order) and a cheap ENGINE op right before
            # the trigger carries the real sem wait -- the trigger itself
            # is sequencer-only and may not legally carry an inline wait.
            wr = st.pop("table_writer", None)
            prep_ins = st.pop("prep", None)
            if wr is not None and prep_ins is not None \
                    and prep_ins.try_remove_dependency(wr):
                gate = nc.gpsimd.memset(gate_sb[:, :1], 0.0)
                DI = bass_rust.DependencyInfo
                prep_ins.add_dependency(wr, DI.NO_SYNC_ONLY)
                gate.ins.add_dependency(wr, DI.SYNC_ONLY)
            nc.gpsimd.trigger_dma(count=None, queue_num=st["trig_q"])

        def emit_group_compute(li, q, si, st):
            L = layers[li]
            sg = pl.sgs[si]
            last3 = li == 2
            last_pass = q == NQ - 1
            tiles, mks = st["tiles"], st["mks"]
            if "dma_wait" in st:
                # explicit gather-completion gate for this group's matmuls
                qn, v = st.pop("dma_wait")
                nc.tensor.wait_ge(dma_sems[qn], v)
            for b in sg["blocks"]:
                my = []
                for (s, qq, sl, cg) in pl.block_chunks[b]:
                    if s != si or qq != q:
                        continue
                    my.append((tiles[0], sl, cg))
                nch = len(my)
                has_lo = NQ == 2 and pl.nchunk_bq[b][0] > 0

                if not last_pass:
                    # lo pass: partial segment sum -> m_lo[b]
                    if nch == 0:
                        continue
                    mps = psA.tile([P, KB, P], F32, tag="acc")
                    nk = 1 if last3 else KB
                    for k in range(nk):
                        for ci, (gt, sl, cg) in enumerate(my):
                            nc.tensor.matmul(
                                mps[:, k, :],
                                gt[:, sl, k * P:(k + 1) * P],
                                s_sb[:, cg, :],
                                start=(ci == 0), stop=(ci == nch - 1))
                    nc.vector.tensor_copy(m_lo[b][:, :nk, :],
                                          mps[:, :nk, :])
                    continue

                bsl = slice(b * P, (b + 1) * P)
                if not last3:
                    # segment mean (transposed): meanT[feat,dst], re-adding
                    # the lo-pass partial via an identity matmul
                    mps = psA.tile([P, KB, P], F32, tag="acc")
                    nops = nch + (1 if has_lo else 0)
                    for k in range(KB):
                        for ci, (gt, sl, cg) in enumerate(my):
                            nc.tensor.matmul(
                                mps[:, k, :],
                                gt[:, sl, k * P:(k + 1) * P],
                                s_sb[:, cg, :],
                                start=(ci == 0), stop=(ci == nops - 1))
                        if has_lo:
                            nc.tensor.matmul(
                                mps[:, k, :], ident_b, m_lo[b][:, k, :],
                                start=(nch == 0), stop=True)
                    m_sb = work.tile([P, KB, P], BF16_T, tag="msb")
                    if nch == 0 and not has_lo:
                        nc.vector.memset(m_sb[:], 0.0)
                    else:
                        # fold the 1/deg mean scaling into the
                        # PSUM->SBUF copy (S is a 0/1 one-hot)
                        i0, i1 = bass.broadcast_tensor_aps(
                            mps[:], idg_sb[:, b:b + 1, :])
                        nc.vector.tensor_tensor(
                            out=m_sb[:], in0=i0, in1=i1,
                            op=mybir.AluOpType.mult)

                    # dense: outT[feat_out, node] = Wl@meanT + Wr@hT
                    ops = psB.tile([P, KB, P], F32, tag="out")
                    for bank in range(KB):
                        for k in range(KB):
                            nc.tensor.matmul(
                                ops[:, bank, :],
                                L["wl"][:, k, bank * P:(bank + 1) * P],
                                m_sb[:, k, :],
                                start=(k == 0), stop=False)
                        for k in range(KB):
                            nc.tensor.matmul(
                                ops[:, bank, :],
                                L["wr"][:, k, bank * P:(bank + 1) * P],
                                ht[b][:, k, :],
                                start=False, stop=(k == KB - 1))

                    # epilogue: tanh(+bias), dropout mask, update hT
                    mk_t, mk_b0 = mks
                    mk = mk_t[:, :, (b - mk_b0) * P:(b - mk_b0 + 1) * P]
                    a_sb = work.tile([P, KB, P], BF16_T, tag="act")
                    if zero_bias:
                        nc.scalar.activation(a_sb[:], ops[:], Tanh)
                    else:
                        for bank in range(KB):
                            nc.scalar.activation(
                                a_sb[:, bank, :], ops[:, bank, :], Tanh,
                                bias=bias_sb[:, L["bcol"] + bank:
                                             L["bcol"] + bank + 1])
                    nc.vector.tensor_mul(
                        out=ht[b][:], in0=a_sb[:], in1=mk)

                    row0 = b * P
                    if li == 0:
                        # node-major copy for AllGather input
                        nm = work.tile([P, D], FP8_T, tag="nm")
                        for bank in range(KB):
                            tp = psT.tile([P, P], BF16_T, tag="tp")
                            nc.tensor.transpose(
                                tp, ht[b][:, bank, :], ident_b)
                            nc.vector.tensor_copy(
                                nm[:, bank * P:(bank + 1) * P], tp)
                        nc.sync.dma_start(cc1[row0:row0 + P, :], nm)

                    if li == 1:
                        # p = h2 @ W3l^T (node-major) for layer-3 gather
                        pp = psP.tile([P, DO], F32, tag="pp")
                        for k in range(KB):
                            nc.tensor.matmul(
                                pp, ht[b][:, k, :], w3l[:, k, :],
                                start=(k == 0), stop=(k == KB - 1))
                        p_sb = work.tile([P, DO], BF16_T, tag="pnm")
                        nc.vector.tensor_copy(p_sb, pp)
                        nc.sync.dma_start(ccp[row0:row0 + P, :], p_sb)

                    # layer boundary AllGather, split in two chunks so
                    # chunk a overlaps the tail of this layer's compute
                    # and chunk b hides behind the next layer's lo pass
                    cin = cc1 if li == 0 else ccp
                    cparts = h1_parts if li == 0 else p_parts
                    if b == pl.S1B - 1:
                        ag = nc.gpsimd.collective_compute(
                            "AllGather", mybir.AluOpType.bypass,
                            replica_groups=rg,
                            ins=[cin[0:r0, :]], outs=[cparts[0].opt()])
                        ag_names[(li + 1, 0)] = ag.ins.name
                    if b == NB - 1 and pl.NPIECE == 2:
                        ag = nc.gpsimd.collective_compute(
                            "AllGather", mybir.AluOpType.bypass,
                            replica_groups=rg,
                            ins=[cin[r0:NPC, :]], outs=[cparts[1].opt()])
                        ag_names[(li + 1, 1)] = ag.ins.name
                else:
                    # layer 3: outT = mean(p)^T + W3r @ hT, tanh, output
                    # (segment sum in its own accumulator so the 1/deg
                    # scaling can apply before joining the dense term)
                    any_agg = nch > 0 or has_lo
                    if any_agg:
                        mps3 = psA.tile([P, KB, P], F32, tag="acc")
                        m3 = mps3[:, 0, :]
                        nops = nch + (1 if has_lo else 0)
                        for ci, (gt, sl, cg) in enumerate(my):
                            nc.tensor.matmul(
                                m3, gt[:, sl, :], s_sb[:, cg, :],
                                start=(ci == 0), stop=(ci == nops - 1))
                        if has_lo:
                            nc.tensor.matmul(
                                m3, ident_b, m_lo[b][:, 0, :],
                                start=(nch == 0), stop=True)
                        m3_sb = work.tile([P, P], BF16_T, tag="m3")
                        nc.vector.tensor_tensor(
                            out=m3_sb, in0=m3, in1=idg_sb[:, b, :],
                            op=mybir.AluOpType.mult)
                    ops = psB.tile([P, KB, P], F32, tag="out")
                    o3 = ops[:, 0, :]
                    for k in range(KB):
                        nc.tensor.matmul(
                            o3, L["wr3"][:, k, :], ht[b][:, k, :],
                            start=(k == 0),
                            stop=(not any_agg and k == KB - 1))
                    if any_agg:
                        nc.tensor.matmul(
                            o3, ident_b, m3_sb, start=False, stop=True)
                    o_sb = work.tile([P, DO], F32, tag="o3")
                    nc.scalar.activation(
                        o_sb, o3, Tanh,
                        bias=bias_sb[:, L["bcol"]:L["bcol"] + 1])
                    tpf = psP.tile([P, DO], F32, tag="pp")
                    nc.tensor.transpose(tpf, o_sb, ident_f)
                    onm = work.tile([P, DO], F32, tag="onm")
                    nc.vector.tensor_copy(onm, tpf)
                    nc.sync.dma_start(out_d[bsl, :], onm)

        # a group's TRIGGER may only be emitted after the compute group that
        # launches the AllGather chunk its gather reads from (the collective
        # must exist in program order for Tile to attach the data dep); the
        # PREP has no such constraint. Preps also must not run more than
        # BMARGIN groups past the triggers, or a prep's buffer-reuse wait
        # could deadlock against a not-yet-emitted trigger.
        si_ag = next(si for si, sg in enumerate(pl.sgs)
                     if pl.S1B - 1 in sg["blocks"])
        gidx = {g: i for i, g in enumerate(groups)}
        req = []
        for (li, q, si) in groups:
            if li == 0:
                req.append(-1)
            else:
                tgt = NSG - 1 if (q == NQ - 1 or NQ == 1) else si_ag
                req.append(gidx[(li - 1, NQ - 1, tgt)])

        # loads (incl. gather preps) are req-gated: the prep must follow the
        # AllGather writer in program order or Tile cannot attach the data
        # dep. Triggers trail the preps by BMARGIN groups so that when a
        # trigger blocks the in-order GpSimd queue waiting for its AllGather,
        # BMARGIN preps' descriptor emission has already run ahead of it.
        states = {}
        load_cursor = 0
        trig_cursor = 0
        BMARGIN = g_bufs - 1
        NG = len(groups)
        for gi in range(NG):
            target = min(gi + LOOK, NG - 1)
            while True:
                if load_cursor <= target and req[load_cursor] <= gi - 1:
                    states[load_cursor] = emit_group_loads(
                        *groups[load_cursor])
                    load_cursor += 1
                    continue
                if trig_cursor < load_cursor and (
                        load_cursor - trig_cursor > BMARGIN
                        or trig_cursor <= gi):
                    emit_group_trigger(states[trig_cursor])
                    trig_cursor += 1
                    continue
                break
            assert trig_cursor > gi, (gi, trig_cursor, load_cursor)
            emit_group_compute(*groups[gi], states.pop(gi))

    nc.compile()
    return nc


# ----------------------------------------------------------------------------
# Host driver
# ----------------------------------------------------------------------------

def prepare(x, edge_index, mask1, mask2,
            W1l, b1, W1r, W2l, b2, W2r, W3l, b3, W3r, gmax=24):
    N, D = x.shape
    DO = W3l.shape[0]
    E = edge_index.shape[1]
    pl = make_plan(N, E, D, DO, edge_index, gmax=gmax)
    KB = pl.KB

    x_bf = x.astype(BF16)
    scale1 = ((mask1 > DROP_P) / (1.0 - DROP_P)).astype(np.float32)
    scale2 = ((mask2 > DROP_P) / (1.0 - DROP_P)).astype(np.float32)

    def packw(W):
        return np.ascontiguousarray(
            W.T.reshape(KB, P, W.shape[0]).transpose(1, 0, 2)).astype(BF16)

    bias = np.zeros((P, 2 * KB + 1), dtype=np.float32)
    for k in range(KB):
        bias[:, k] = b1[k * P:(k + 1) * P]
        bias[:, KB + k] = b2[k * P:(k + 1) * P]
    bias[:, 2 * KB] = b3[:P]

    shared = {
        "w1l": packw(W1l), "w1r": packw(W1r),
        "w2l": packw(W2l), "w2r": packw(W2r),
        "w3l": packw(W3l), "w3r": packw(W3r),
        "bias": bias,
    }
    in_maps = []
    for c in range(NCORES):
        m = dict(shared)
        m.update(pack_core(pl, c, x_bf, scale1, scale2))
        in_maps.append(m)
    return pl, in_maps


def kernel(x, edge_index, mask1, mask2,
           W1l, b1, W1r, W2l, b2, W2r, W3l, b3, W3r):
    x = np.asarray(x, dtype=np.float32)
    pl, in_maps = prepare(
        x, np.asarray(edge_index),
        np.asarray(mask1, dtype=np.float32),
        np.asarray(mask2, dtype=np.float32),
        np.asarray(W1l, np.float32), np.asarray(b1, np.float32),
        np.asarray(W1r, np.float32),
        np.asarray(W2l, np.float32), np.asarray(b2, np.float32),
        np.asarray(W2r, np.float32),
        np.asarray(W3l, np.float32), np.asarray(b3, np.float32),
        np.asarray(W3r, np.float32))
    zb = (np.abs(b1).max() == 0 and np.abs(b2).max() == 0
          and np.abs(b3).max() == 0)
    nc = build_program(pl, zero_bias=bool(zb))
    res = run_bass_kernel_spmd(nc, in_maps, core_ids=list(range(NCORES)))
    N = x.shape[0]
    out = np.zeros((N, pl.DO), dtype=np.float32)
    for c in range(NCORES):
        lo, hi = c * pl.NPC, min((c + 1) * pl.NPC, N)
        out[lo:hi] = res.results[c]["out"][:hi - lo]
    return out

